# revision 38
# baseline (speedup 1.0000x reference)
"""Bass/Trainium2 kernel for the BarlowTwins-style cross-entropy loss.

Reference (per batch b of 8):
    logits = z1[b].T @ z2[b] / T            (2048 x 2048, K=256, T=1.0)
    logp   = log_softmax(logits, axis=0)
    loss   = -mean_b,m logp[m, m]

Sharding: pure data parallel over batch b -> one element per NeuronCore.
Each core computes logitsT[m, n] = sum_s z2[s,m]*z1[s,n] in 16 row-blocks
of [128 m x 2048 n] (one [128,2048] PSUM tile each, double-buffered) so the
softmax reduction runs along the free axis.

Per-row shift: logZ[m] needs exp(x - c_m) with c_m within ~85 nats of the
row max.  c_m = max(diag[m], 60) + 40 with diag computed on host (needed
for the loss anyway; diag is an entry of row m, so c_m <= rowmax + 80 ->
no underflow).  Rows whose max exceeds c_m + ~85 overflow to inf/NaN on
device; the host detects and recomputes those exactly (~1.8% here).

Engines:
  - PE: fp8 DoubleRow matmuls, K=256 in one MM, 4 x 512-wide per block.
  - Every block is consumed by BOTH exp engines concurrently on the same
    PSUM tile:
      ACT strip [0:WA):    exp with bias=-c_m, accum_out -> sea column.
      DVE strip [WA:2048) (Schraudolph): uint16 bits = round(x*184.665 +
        (16256 - 184.665*c_m)); the f32->uint16 convert saturates both
        ways (HW-verified: negative -> 0 -> bf16 +0.0; huge -> NaN/inf so
        overflow stays detectable).  The uint16 tile bitcast to bf16 IS
        e^(x-c_m) to ~3%; a 1x reduce sums it.
    Host adds the two partials (same c_m).  WA=1536 keeps the strips
    bank-aligned (ACT banks 0-2, DVE bank 3 -- same-bank concurrent reads
    are legal but ~20% slower, measured).  Schraudolph's ~1% bias on a
    quarter of the columns shifts the loss ~4e-3 relative; gate is 2e-2.

Startup: inputs are chunked into separate tiles (precise DMA deps), spread
over the sync + scalar DGE queues, first-needed first; the first blocks'
matmuls double as the HAM warm-up.

Host merge: logZ = c_m + log(sea + sed); loss = -mean(diag - logZ).
"""

import numpy as np
import ml_dtypes

import concourse.bass as bass
import concourse.tile as tile
from concourse import bacc, mybir
from concourse.bass_utils import run_bass_kernel_spmd

B = 8          # batch (one element per core)
S = 256        # contraction dim
N = 2048       # feature dim (n and m)
P = 128        # SBUF partitions
NBLK = N // P  # 16 row blocks
NQ = 4         # 512-wide psum quarters per block

A_SCH = 128.0 / float(np.log(2.0))   # Schraudolph scale (bf16 bits/nat)
WA = 1536                            # ACT strip width (DVE takes N - WA)
# Bank-aligned: ACT reads PSUM banks 0-2, DVE reads bank 3 -- concurrent
# reads of the SAME bank are legal but ~20% slower (measured); avoid.

_CACHE = {}


def _build(wa=WA):
    key = ("nc", wa)
    if key in _CACHE:
        return _CACHE[key]
    wd = N - wa

    f32 = mybir.dt.float32
    bf16 = mybir.dt.bfloat16
    u16 = mybir.dt.uint16
    fp8 = mybir.dt.float8e4

    nc = bacc.Bacc("TRN2", target_bir_lowering=False, debug=False)
    # z1: moving operand, [128, q, ktile, 512] (q-major: rhs slices are
    # contiguous); z2: weights, [128, blk, ktile, 128].
    z1 = nc.dram_tensor("z1", [P, NQ, 2, 512], fp8, kind="ExternalInput").ap()
    z2 = nc.dram_tensor("z2", [P, NBLK, 2, P], fp8, kind="ExternalInput").ap()
    # cb packs -c_m (cols 0:16) and the Schraudolph bias (cols 16:32).
    cb = nc.dram_tensor("cb", [P, 2 * NBLK], f32, kind="ExternalInput").ap()
    sea_d = nc.dram_tensor("sea", [P, NBLK], f32, kind="ExternalOutput").ap()
    sed_d = nc.dram_tensor("sed", [P, NBLK], f32, kind="ExternalOutput").ap()

    with tile.TileContext(nc) as tc:
        with (
            tc.tile_pool(name="sb", bufs=1) as cpool,
            tc.tile_pool(name="ps", bufs=2, space="PSUM") as papool,
        ):
            zpool = cpool
            tpool = cpool
            spool = cpool
            pdpool = papool
            # Input tiles, one per DMA so dependencies stay precise.
            z1A = zpool.tile([P, 2, 2, 512], fp8, tag="z1A")       # quarters 0-1
            z1B = zpool.tile([P, 2, 2, 512], fp8, tag="z1B")       # quarters 2-3
            z2a = zpool.tile([P, 2, 2, P], fp8, tag="z2a")         # blocks 0-1
            z2b = zpool.tile([P, NBLK - 2, 2, P], fp8, tag="z2b")  # blocks 2-15
            cbt = cpool.tile([P, 2 * NBLK], f32, tag="cbt")

            # PE warm-up with no DMA dependency: junk bf16 matmuls on a
            # memset SBUF tile keep the PE busy ~3.4us so the HAM clock
            # gate opens before the real MM stream begins.
            warm = cpool.tile([P, 512], bf16, tag="warm")
            nc.gpsimd.memset(warm[:], 0.0)
            jp = papool.tile([P, wa], f32, tag="pa")
            for _ in range(8):
                nc.tensor.matmul(
                    jp[:, 0:512],
                    lhsT=warm[:, 0:P],
                    rhs=warm[:],
                    start=True,
                    stop=True,
                )

            # Scalar DGE queue: bias consts + z1 quarters 2-3 + bulk z2.
            nc.scalar.dma_start(z1B[:], z1[:, 2:4])
            nc.scalar.dma_start(cbt[:], cb)
            nc.scalar.dma_start(z2b[:], z2[:, 2:NBLK])
            # Sync DGE queue: first-needed operands first.
            nc.sync.dma_start(z1A[:], z1[:, 0:2])
            nc.sync.dma_start(z2a[:], z2[:, 0:2])

            # ACT exp-table preload, overlapped with the input DMAs.
            dummy = cpool.tile([1, 1], f32, tag="dummy")
            nc.scalar.memzero(dummy[:])
            nc.scalar.activation(
                dummy[:], dummy[:], mybir.ActivationFunctionType.Exp,
                bias=dummy[0:1, 0:1],
            )

            # Outputs; each engine has its own staging tile.  Every
            # column is written by its block's accum/reduce, so no init.
            sea = cpool.tile([P, NBLK], f32, tag="sea")
            sed = cpool.tile([P, NBLK], f32, tag="sed")

            for blk in range(NBLK):
                w = z2a[:, blk] if blk < 2 else z2b[:, blk - 2]
                pa = papool.tile([P, wa], f32, tag="pa")
                pd = pdpool.tile([P, wd], f32, tag="pd")
                for q in range(3):
                    rhs = z1A[:, q] if q < 2 else z1B[:, q - 2]
                    nc.tensor.matmul(
                        pa[:, q * 512 : (q + 1) * 512],
                        lhsT=w,
                        rhs=rhs,
                        start=True,
                        stop=True,
                        perf_mode=mybir.MatmulPerfMode.DoubleRow,
                    )
                nc.tensor.matmul(
                    pd[:],
                    lhsT=w,
                    rhs=z1B[:, 1],
                    start=True,
                    stop=True,
                    perf_mode=mybir.MatmulPerfMode.DoubleRow,
                )
                # DVE strip: Schraudolph bits, then reduce.
                ut = spool.tile([P, wd], u16, tag="ut", bufs=3)
                nc.vector.tensor_scalar(
                    ut[:],
                    pd[:],
                    A_SCH,
                    cbt[:, NBLK + blk : NBLK + blk + 1],
                    op0=mybir.AluOpType.mult,
                    op1=mybir.AluOpType.add,
                )
                KEEPWARM = blk >= 3 and blk % 2 == 1 and blk < 15
                if KEEPWARM:
                    # Anti-throttle filler: one junk MM in the PE's idle
                    # slack (into the already-consumed pd tile) keeps the
                    # HAM activity window busy so the PE never re-throttles
                    # mid-run.
                    nc.tensor.matmul(
                        pd[:],
                        lhsT=warm[:, 0:P],
                        rhs=warm[:],
                        start=True,
                        stop=True,
                    )
                nc.vector.tensor_reduce(
                    sed[:, blk : blk + 1],
                    ut[:].bitcast(bf16),
                    axis=mybir.AxisListType.X,
                    op=mybir.AluOpType.add,
                )
                # ACT strip: exp(x - c_m) accumulated along the row.
                trash = tpool.tile([P, wa], bf16, tag="trash", bufs=3)
                nc.scalar.activation(
                    trash[:],
                    pa[:],
                    mybir.ActivationFunctionType.Exp,
                    bias=cbt[:, blk : blk + 1],
                    scale=1.0,
                    accum_out=sea[:, blk : blk + 1],
                )
                if blk == NBLK - 2:
                    # Most of the output goes out while the last block runs
                    # (sync queue: keeps the ACT queue free of DMA dispatch).
                    nc.sync.dma_start(sea_d[:, 0 : NBLK - 1], sea[:, 0 : NBLK - 1])
                    nc.sync.dma_start(sed_d[:, 0 : NBLK - 1], sed[:, 0 : NBLK - 1])

            nc.sync.dma_start(sea_d[:, NBLK - 1 : NBLK], sea[:, NBLK - 1 : NBLK])
            nc.sync.dma_start(sed_d[:, NBLK - 1 : NBLK], sed[:, NBLK - 1 : NBLK])

    nc.compile()
    _CACHE[key] = nc
    return nc


def _prep(z1, z2):
    """Host-side packing: fp8 + DoubleRow interleave + per-row bias."""
    z1 = np.ascontiguousarray(z1, dtype=np.float32)
    z2 = np.ascontiguousarray(z2, dtype=np.float32)
    dg64 = np.einsum("bsm,bsm->bm", z1, z2, dtype=np.float64)
    c = (np.maximum(dg64.astype(np.float32), 60.0) + 40.0).astype(np.float32)

    z1f = z1.astype(ml_dtypes.float8_e4m3)
    z2f = z2.astype(ml_dtypes.float8_e4m3)

    in_maps = []
    for b in range(B):
        # [s, n] -> [p, q, ktile, 512] with s = ktile*128 + p
        z1b = np.ascontiguousarray(
            z1f[b].reshape(2, P, NQ, 512).transpose(1, 2, 0, 3)
        )
        z2b = np.ascontiguousarray(
            z2f[b].reshape(2, P, NBLK, P).transpose(1, 2, 0, 3)
        )
        cbm = c[b].reshape(NBLK, P).T  # [p, blk], m = blk*128 + p
        cbt = np.concatenate(
            [-cbm, 16256.0 - A_SCH * cbm], axis=1
        ).astype(np.float32)
        in_maps.append({"z1": z1b, "z2": z2b, "cb": np.ascontiguousarray(cbt)})
    return z1, z2, dg64, c, in_maps


def _run(z1, z2, wa=WA, **spmd_kwargs):
    nc = _build(wa)
    z1, z2, dg64, c, in_maps = _prep(z1, z2)
    res = run_bass_kernel_spmd(nc, in_maps, core_ids=list(range(B)), **spmd_kwargs)

    total = 0.0
    for b in range(B):
        sea = res.results[b]["sea"].astype(np.float64)  # [p, blk]
        sed = res.results[b]["sed"].astype(np.float64)
        se_m = (sea + sed).T.reshape(N)  # m = blk*128 + p
        cb = c[b].astype(np.float64)
        bad = ~np.isfinite(se_m) | (se_m <= 0.0)
        with np.errstate(divide="ignore", invalid="ignore"):
            logZ = cb + np.log(se_m)
        if bad.any():
            idx = np.where(bad)[0]
            rows = z2[b][:, idx].T.astype(np.float64) @ z1[b].astype(np.float64)
            m0 = rows.max(axis=1)
            logZ[idx] = m0 + np.log(np.exp(rows - m0[:, None]).sum(axis=1))
        total += (dg64[b] - logZ).sum()
    loss = -total / (B * N)
    return np.asarray(loss, dtype=np.float32), res


def kernel(z1, z2):
    loss, _ = _run(z1, z2)
    return loss


# revision 39
# speedup vs baseline: 1.0313x; 1.0313x over previous
"""Bass/Trainium2 kernel for the BarlowTwins-style cross-entropy loss.

Reference (per batch b of 8):
    logits = z1[b].T @ z2[b] / T            (2048 x 2048, K=256, T=1.0)
    logp   = log_softmax(logits, axis=0)
    loss   = -mean_b,m logp[m, m]

Sharding: pure data parallel over batch b -> one element per NeuronCore.
Each core computes logitsT[m, n] = sum_s z2[s,m]*z1[s,n] in 16 row-blocks
of [128 m x 2048 n] (one [128,2048] PSUM tile each, double-buffered) so the
softmax reduction runs along the free axis.

Per-row shift: logZ[m] needs exp(x - c_m) with c_m within ~85 nats of the
row max.  c_m = max(diag[m], 60) + 40 with diag computed on host (needed
for the loss anyway; diag is an entry of row m, so c_m <= rowmax + 80 ->
no underflow).  Rows whose max exceeds c_m + ~85 overflow to inf/NaN on
device; the host detects and recomputes those exactly (~1.8% here).

Engines:
  - PE: fp8 DoubleRow matmuls, K=256 in one MM, 4 x 512-wide per block.
  - Every block is consumed by BOTH exp engines concurrently on the same
    PSUM tile:
      ACT strip [0:WA):    exp with bias=-c_m, accum_out -> sea column.
      DVE strip [WA:2048) (Schraudolph): uint16 bits = round(x*184.665 +
        (16256 - 184.665*c_m)); the f32->uint16 convert saturates both
        ways (HW-verified: negative -> 0 -> bf16 +0.0; huge -> NaN/inf so
        overflow stays detectable).  The uint16 tile bitcast to bf16 IS
        e^(x-c_m) to ~3%; a 1x reduce sums it.
    Host adds the two partials (same c_m).  WA=1536 keeps the strips
    bank-aligned (ACT banks 0-2, DVE bank 3 -- same-bank concurrent reads
    are legal but ~20% slower, measured).  Schraudolph's ~1% bias on a
    quarter of the columns shifts the loss ~4e-3 relative; gate is 2e-2.

Startup: inputs are chunked into separate tiles (precise DMA deps), spread
over the sync + scalar DGE queues, first-needed first; the first blocks'
matmuls double as the HAM warm-up.

Host merge: logZ = c_m + log(sea + sed); loss = -mean(diag - logZ).
"""

import numpy as np
import ml_dtypes

import concourse.bass as bass
import concourse.tile as tile
from concourse import bacc, mybir
from concourse.bass_utils import run_bass_kernel_spmd

B = 8          # batch (one element per core)
S = 256        # contraction dim
N = 2048       # feature dim (n and m)
P = 128        # SBUF partitions
NBLK = N // P  # 16 row blocks
NQ = 4         # 512-wide psum quarters per block

A_SCH = 128.0 / float(np.log(2.0))   # Schraudolph scale (bf16 bits/nat)
WA = 1536                            # ACT strip width (DVE takes N - WA)
# Bank-aligned: ACT reads PSUM banks 0-2, DVE reads bank 3 -- concurrent
# reads of the SAME bank are legal but ~20% slower (measured); avoid.

_CACHE = {}


def _build(wa=WA):
    key = ("nc", wa)
    if key in _CACHE:
        return _CACHE[key]
    wd = N - wa

    f32 = mybir.dt.float32
    bf16 = mybir.dt.bfloat16
    u16 = mybir.dt.uint16
    fp8 = mybir.dt.float8e4

    nc = bacc.Bacc("TRN2", target_bir_lowering=False, debug=False)
    # z1: moving operand, [128, q, ktile, 512] (q-major: rhs slices are
    # contiguous); z2: weights, [128, blk, ktile, 128].
    z1 = nc.dram_tensor("z1", [P, NQ, 2, 512], fp8, kind="ExternalInput").ap()
    z2 = nc.dram_tensor("z2", [P, NBLK, 2, P], fp8, kind="ExternalInput").ap()
    # cb packs -c_m (cols 0:16) and the Schraudolph bias (cols 16:32).
    cb = nc.dram_tensor("cb", [P, 2 * NBLK], f32, kind="ExternalInput").ap()
    sea_d = nc.dram_tensor("sea", [P, NBLK], f32, kind="ExternalOutput").ap()
    sed_d = nc.dram_tensor("sed", [P, NBLK], f32, kind="ExternalOutput").ap()

    with tile.TileContext(nc) as tc:
        with (
            tc.tile_pool(name="sb", bufs=1) as cpool,
            tc.tile_pool(name="ps", bufs=2, space="PSUM") as papool,
        ):
            zpool = cpool
            tpool = cpool
            spool = cpool
            pdpool = papool
            # Input tiles, one per DMA so dependencies stay precise.
            z1A = zpool.tile([P, 2, 2, 512], fp8, tag="z1A")       # quarters 0-1
            z1B = zpool.tile([P, 2, 2, 512], fp8, tag="z1B")       # quarters 2-3
            z2a = zpool.tile([P, 2, 2, P], fp8, tag="z2a")         # blocks 0-1
            z2b = zpool.tile([P, NBLK - 2, 2, P], fp8, tag="z2b")  # blocks 2-15
            cbt = cpool.tile([P, 2 * NBLK], f32, tag="cbt")

            # PE warm-up with no DMA dependency: junk bf16 matmuls on a
            # memset SBUF tile keep the PE busy ~3.4us so the HAM clock
            # gate opens before the real MM stream begins.
            warm = cpool.tile([P, 512], bf16, tag="warm")
            nc.gpsimd.memset(warm[:], 0.0)
            jp = papool.tile([P, wa], f32, tag="pa")
            for _ in range(8):
                nc.tensor.matmul(
                    jp[:, 0:512],
                    lhsT=warm[:, 0:P],
                    rhs=warm[:],
                    start=True,
                    stop=True,
                )

            # Scalar DGE queue: bias consts + z1 quarters 2-3 + bulk z2.
            nc.scalar.dma_start(z1B[:], z1[:, 2:4])
            nc.scalar.dma_start(cbt[:], cb)
            nc.scalar.dma_start(z2b[:], z2[:, 2:NBLK])
            # Sync DGE queue: first-needed operands first.
            nc.sync.dma_start(z1A[:], z1[:, 0:2])
            nc.sync.dma_start(z2a[:], z2[:, 0:2])

            # ACT exp-table preload, overlapped with the input DMAs.
            dummy = cpool.tile([1, 1], f32, tag="dummy")
            nc.scalar.memzero(dummy[:])
            nc.scalar.activation(
                dummy[:], dummy[:], mybir.ActivationFunctionType.Exp,
                bias=dummy[0:1, 0:1],
            )

            # Outputs; each engine has its own staging tile.  Every
            # column is written by its block's accum/reduce, so no init.
            sea = cpool.tile([P, NBLK], f32, tag="sea")
            sed = cpool.tile([P, NBLK], f32, tag="sed")

            for blk in range(NBLK):
                w = z2a[:, blk] if blk < 2 else z2b[:, blk - 2]
                pa = papool.tile([P, wa], f32, tag="pa")
                pd = pdpool.tile([P, wd], f32, tag="pd")
                for q in range(3):
                    rhs = z1A[:, q] if q < 2 else z1B[:, q - 2]
                    nc.tensor.matmul(
                        pa[:, q * 512 : (q + 1) * 512],
                        lhsT=w,
                        rhs=rhs,
                        start=True,
                        stop=True,
                        perf_mode=mybir.MatmulPerfMode.DoubleRow,
                    )
                nc.tensor.matmul(
                    pd[:],
                    lhsT=w,
                    rhs=z1B[:, 1],
                    start=True,
                    stop=True,
                    perf_mode=mybir.MatmulPerfMode.DoubleRow,
                )
                # DVE strip: Schraudolph bits, then reduce.
                ut = spool.tile([P, wd], u16, tag="ut", bufs=3)
                nc.vector.tensor_scalar(
                    ut[:],
                    pd[:],
                    A_SCH,
                    cbt[:, NBLK + blk : NBLK + blk + 1],
                    op0=mybir.AluOpType.mult,
                    op1=mybir.AluOpType.add,
                )
                nc.vector.tensor_reduce(
                    sed[:, blk : blk + 1],
                    ut[:].bitcast(bf16),
                    axis=mybir.AxisListType.X,
                    op=mybir.AluOpType.add,
                )
                # ACT strip: exp(x - c_m) accumulated along the row.
                trash = tpool.tile([P, wa], bf16, tag="trash", bufs=3)
                nc.scalar.activation(
                    trash[:],
                    pa[:],
                    mybir.ActivationFunctionType.Exp,
                    bias=cbt[:, blk : blk + 1],
                    scale=1.0,
                    accum_out=sea[:, blk : blk + 1],
                )
                if blk == NBLK - 2:
                    # Most of the output goes out while the last block runs
                    # (sync queue: keeps the ACT queue free of DMA dispatch).
                    nc.sync.dma_start(sea_d[:, 0 : NBLK - 1], sea[:, 0 : NBLK - 1])
                    nc.sync.dma_start(sed_d[:, 0 : NBLK - 1], sed[:, 0 : NBLK - 1])

            nc.sync.dma_start(sea_d[:, NBLK - 1 : NBLK], sea[:, NBLK - 1 : NBLK])
            nc.sync.dma_start(sed_d[:, NBLK - 1 : NBLK], sed[:, NBLK - 1 : NBLK])

    nc.compile()
    _CACHE[key] = nc
    return nc


def _prep(z1, z2):
    """Host-side packing: fp8 + DoubleRow interleave + per-row bias."""
    z1 = np.ascontiguousarray(z1, dtype=np.float32)
    z2 = np.ascontiguousarray(z2, dtype=np.float32)
    dg64 = np.einsum("bsm,bsm->bm", z1, z2, dtype=np.float64)
    c = (np.maximum(dg64.astype(np.float32), 60.0) + 40.0).astype(np.float32)

    z1f = z1.astype(ml_dtypes.float8_e4m3)
    z2f = z2.astype(ml_dtypes.float8_e4m3)

    in_maps = []
    for b in range(B):
        # [s, n] -> [p, q, ktile, 512] with s = ktile*128 + p
        z1b = np.ascontiguousarray(
            z1f[b].reshape(2, P, NQ, 512).transpose(1, 2, 0, 3)
        )
        z2b = np.ascontiguousarray(
            z2f[b].reshape(2, P, NBLK, P).transpose(1, 2, 0, 3)
        )
        cbm = c[b].reshape(NBLK, P).T  # [p, blk], m = blk*128 + p
        cbt = np.concatenate(
            [-cbm, 16256.0 - A_SCH * cbm], axis=1
        ).astype(np.float32)
        in_maps.append({"z1": z1b, "z2": z2b, "cb": np.ascontiguousarray(cbt)})
    return z1, z2, dg64, c, in_maps


def _run(z1, z2, wa=WA, **spmd_kwargs):
    nc = _build(wa)
    z1, z2, dg64, c, in_maps = _prep(z1, z2)
    res = run_bass_kernel_spmd(nc, in_maps, core_ids=list(range(B)), **spmd_kwargs)

    total = 0.0
    for b in range(B):
        sea = res.results[b]["sea"].astype(np.float64)  # [p, blk]
        sed = res.results[b]["sed"].astype(np.float64)
        se_m = (sea + sed).T.reshape(N)  # m = blk*128 + p
        cb = c[b].astype(np.float64)
        bad = ~np.isfinite(se_m) | (se_m <= 0.0)
        with np.errstate(divide="ignore", invalid="ignore"):
            logZ = cb + np.log(se_m)
        if bad.any():
            idx = np.where(bad)[0]
            rows = z2[b][:, idx].T.astype(np.float64) @ z1[b].astype(np.float64)
            m0 = rows.max(axis=1)
            logZ[idx] = m0 + np.log(np.exp(rows - m0[:, None]).sum(axis=1))
        total += (dg64[b] - logZ).sum()
    loss = -total / (B * N)
    return np.asarray(loss, dtype=np.float32), res


def kernel(z1, z2):
    loss, _ = _run(z1, z2)
    return loss
